# revision 33
# baseline (speedup 1.0000x reference)
"""LIF neuron scan kernel for Trainium2 (8 NeuronCores, raw Bass SPMD).

The LIF recurrence v_t = 0.5*v_{t-1} + x_t + r_t, s_t = (v_t > 0),
v_t *= (1 - s_t) depends on the past only through m_t = min(v_t, 0):

    m_t = min(0.5*m_{t-1} + u_t, 0),   u_t = x_t + r_t
    s_t = (0.5*m_{t-1} + u_t > 0)

Scaling by 2^t (exact in fp) removes the decay: with M_t = 2^t*m_t and
U_t = 2^t*u_t,  M_t = min(M_{t-1} + U_t, 0)  — one slot of DVE
tensor_tensor_scan(op0=add, op1=min) per timestep.  Time is chunked
(K=10 steps/chunk) so the 2^k prescale (folded into the host-side fp16
input encode) stays in fp16 range.  Each partition holds E=128 neurons as
E segments of S=K+1 scan slots:

  U slot 0:     +60000 with d1 slot 0 = carry (fp32): the segment-boundary
                reset and carry insert fuse into one slot, because
                min(state + 60000, carry) = carry  (|state| << 60000,
                carry <= 0).  d1 = 0 at the data slots.
  U slots 1..K: U_k (fp16)

Spikes come from one tensor_scalar (the only two-op form walrus codegen
accepts on Pool):  s_k = (M_k >= 0), uint8 output — exact except when
v_k == 0 exactly (measure-zero on the fp16 input grid); host decodes
u8 -> f32.

Sharding: fully data-parallel along batch; core i gets inp[:, 8i:8i+8, :].
Engines: SP issues DMA (x,r interleaved in one ExternalInput -> one in-DMA
per chunk); DVE does sum (fp16 2x) + carry inject + scan; Pool (gpsimd)
does init memsets and the spike compare.  Buffers are NB=4 deep; the first
and last chunks are split into segment halves to shorten pipeline fill
and drain.
"""
import sys
sys.path.insert(0, "/opt/trn_rl_repo")
import numpy as np
import concourse.bass as bass
from concourse import mybir
from concourse.bass_utils import run_bass_kernel_spmd

F32 = mybir.dt.float32
F16 = mybir.dt.float16
U8 = mybir.dt.uint8

T, B, N = 100, 64, 2048
NCORES = 8
B_LOC = B // NCORES       # 8
BN = B_LOC * N            # 16384 neurons per core
P = 128                   # partitions
E = BN // P               # 128 neurons (segments) per partition
K = 10                    # timesteps per chunk
C = T // K                # 10 chunks
S = K + 1                 # scan slots per segment (slot0 = reset+carry)
EK = E * K                # 1280
NB = 4                    # buffer depth
BIG = 60000.0             # fp16 segment-reset constant
OUT_LAG = 3               # chunks between spike and its out-DMA issue

# per-chunk segment splits: sub-chunk granularity at the pipeline fill
# (chunks 0-1) and drain (last chunk)
_Q = (0, E // 4, E // 2, 3 * E // 4, E)
_H = (0, E // 2, E)
_SPLITS = {0: _Q, 1: _H, C - 1: _Q}


def _tasks_of(c):
    bounds = _SPLITS.get(c, (0, E))
    return [(bounds[i], bounds[i + 1]) for i in range(len(bounds) - 1)]


# in-DMA granularity is decoupled from compute: SP issue overhead (~1.2us
# per DMA) makes small DMAs issue-bound, so only chunk 0 is split (halves)
_IN_SPLITS = {0: _H}


def _in_tasks_of(c):
    bounds = _IN_SPLITS.get(c, (0, E))
    return [(bounds[i], bounds[i + 1]) for i in range(len(bounds) - 1)]


def _build_nc():
    nc = bass.Bass()
    xr_ext = nc.dram_tensor("xr", [C * P, 2 * EK], F16, kind="ExternalInput")
    s_ext = nc.dram_tensor("s", [C * P, EK], U8, kind="ExternalOutput")

    xrv = xr_ext.rearrange("(c p) f -> c p f", c=C, p=P)
    sv = s_ext.rearrange("(c p) f -> c p f", c=C, p=P)

    AOP = mybir.AluOpType

    # cumulative op counts per chunk, shared by all engine programs
    n_tasks = [len(_tasks_of(c)) for c in range(C)]
    scans_thru = np.cumsum(n_tasks)     # scans completed through chunk c
    spks_thru = np.cumsum(n_tasks)      # spikes completed through chunk c
    outs_thru = np.arange(1, C + 1)     # one out-DMA per chunk

    with (
        nc.sbuf_tensor([P, NB, E, 2, K], F16) as xr,
        nc.sbuf_tensor([P, NB, E, S], F16) as Ub,
        nc.sbuf_tensor([P, NB, E, S], F32) as D1,
        nc.sbuf_tensor([P, NB, E, S], F32) as Mb,
        nc.sbuf_tensor([P, NB, E, K], U8) as sb,
        nc.semaphore() as sem_in,
        nc.semaphore() as sem_init,
        nc.semaphore() as sem_inj,
        nc.semaphore() as sem_scan,
        nc.semaphore() as sem_spk,
        nc.semaphore() as sem_out,
        nc.Block() as block,
    ):
        def seg_flat(ap):  # [P, seg, D] -> [P, seg*D]
            return ap.rearrange("p e d -> p (e d)")

        # chunk C-1's spikes run on DVE (drain shortening); Pool handles
        # spikes for chunks 0..C-2 and all carry injects; Act issues every
        # out-DMA so the input stream on SP never blocks on spikes
        pool_spks_thru = spks_thru.copy()

        @block.sync
        def _(sync):
            for c in range(C):
                b = c % NB
                if c >= NB:
                    sync.wait_ge(sem_scan, scans_thru[c - NB])  # xr[b] free
                for (lo, hi) in _in_tasks_of(c):
                    sync.dma_start(
                        xr[:, b, lo:hi, :, :].rearrange("p e two k -> p (e two k)"),
                        xrv[c][:, lo * 2 * K:hi * 2 * K],
                    ).then_inc(sem_in, 16)

        @block.scalar
        def _(scalar):
            for co in range(C - 1):
                b = co % NB
                scalar.wait_ge(sem_spk, spks_thru[co])
                nc.scalar.dma_start(
                    sv[co], seg_flat(sb[:, b]),
                ).then_inc(sem_out, 16)
            # last chunk: two half out-DMAs chasing the DVE spike quarters
            b = (C - 1) % NB
            for (lo, hi) in ((0, E // 2), (E // 2, E)):
                scalar.wait_ge(sem_spk, spks_thru[C - 2] + hi // (E // 4))
                nc.scalar.dma_start(
                    sv[C - 1][:, lo * K:hi * K], seg_flat(sb[:, b, lo:hi]),
                ).then_inc(sem_out, 16)

        in_before = np.concatenate(
            [[0], np.cumsum([len(_in_tasks_of(c)) for c in range(C)])]
        )

        def _in_req(c, hi):
            """in-DMAs that must have landed before segs [0,hi) of chunk c."""
            n = 0
            for (ilo, ihi) in _in_tasks_of(c):
                n += 1
                if ihi >= hi:
                    break
            return int(in_before[c]) + n

        @block.vector
        def _(vector):
            scan_cnt = 0
            spk_cnt = int(spks_thru[C - 2])   # DVE spikes continue the count
            for c in range(C):
                b = c % NB
                if c >= NB:
                    vector.wait_ge(sem_spk, pool_spks_thru[c - NB])  # U/M[b] free
                if c < NB:
                    vector.wait_ge(sem_init, b + 1)    # D1/U[b] memsets done
                if c == C - 1:
                    vector.wait_ge(sem_out, 16 * outs_thru[c - NB])  # sb[b] free
                first = True
                pend_spk = []   # chunk C-1 spikes, interleaved one task behind
                for (lo, hi) in _tasks_of(c):
                    vector.wait_ge(sem_in, 16 * _in_req(c, hi))
                    nc.vector.tensor_tensor(
                        Ub[:, b, lo:hi, 1:S],
                        xr[:, b, lo:hi, 0, :], xr[:, b, lo:hi, 1, :],
                        AOP.add,
                    )
                    if first and c >= 1:
                        vector.wait_ge(sem_inj, c)     # Pool injected carry
                    first = False
                    scan_cnt += 1
                    nc.vector.tensor_tensor_scan(
                        seg_flat(Mb[:, b, lo:hi]), seg_flat(Ub[:, b, lo:hi]),
                        seg_flat(D1[:, b, lo:hi]), 0.0,
                        AOP.add, AOP.min,
                    ).then_inc(sem_scan, 1)
                    if c == C - 1:
                        while pend_spk:
                            plo, phi = pend_spk.pop(0)
                            spk_cnt += 1
                            nc.vector.tensor_scalar(
                                sb[:, b, plo:phi], Mb[:, b, plo:phi, 1:S],
                                0.0, None, AOP.is_ge,
                            ).then_inc(sem_spk, 1)
                        pend_spk.append((lo, hi))
                for (plo, phi) in pend_spk:
                    spk_cnt += 1
                    nc.vector.tensor_scalar(
                        sb[:, b, plo:phi], Mb[:, b, plo:phi, 1:S],
                        0.0, None, AOP.is_ge,
                    ).then_inc(sem_spk, 1)

        @block.gpsimd
        def _(pool):
            for b in range(NB):
                nc.gpsimd.memset(seg_flat(D1[:, b]), 0.0)
                nc.gpsimd.memset(Ub[:, b, :, 0:1], BIG).then_inc(sem_init, 1)
            scan_cnt = 0
            for c in range(C - 1):
                b = c % NB
                tasks = _tasks_of(c)
                for ti, (lo, hi) in enumerate(tasks):
                    scan_cnt += 1
                    pool.wait_ge(sem_scan, scan_cnt)
                    if ti == len(tasks) - 1:
                        # inject chunk c+1's carry (reads all of M[b], writes
                        # D1[(c+1)%NB] slot0) before this task's spike
                        nc.gpsimd.tensor_scalar(
                            D1[:, (c + 1) % NB, :, 0:1], Mb[:, b, :, S - 1:S],
                            float(2.0 ** -K), None, AOP.mult,
                        ).then_inc(sem_inj, 1)
                    if c >= NB:
                        pool.wait_ge(sem_out, 16 * outs_thru[c - NB])
                    nc.gpsimd.tensor_scalar(
                        sb[:, b, lo:hi], Mb[:, b, lo:hi, 1:S],
                        0.0, None, AOP.is_ge,
                    ).then_inc(sem_spk, 1)

    return nc


_POW2 = (2.0 ** np.arange(K)).astype(np.float32)


def _encode(inp: np.ndarray, rec: np.ndarray):
    """Full [T,B,N] f32 pair -> per-core [C*P, 2*EK] f16 interleaved arrays."""
    x = inp.reshape(T, B * N)
    r = rec.reshape(T, B * N)
    sc = _POW2[None, :, None]
    xs = (x.reshape(C, K, B * N) * sc).astype(np.float16)
    # encode r against the ideal single-rounded fp16 target of (x+r)*2^k so
    # the on-device fp16 sum xs+rs lands on (or next to) that target
    target = ((x + r).reshape(C, K, B * N) * sc).astype(np.float16)
    rs = (target.astype(np.float32) - xs.astype(np.float32)).astype(np.float16)
    maps = []
    for i in range(NCORES):
        parts = []
        for src in (xs, rs):
            a = src.reshape(C, K, B, N)[:, :, i * B_LOC:(i + 1) * B_LOC, :]
            parts.append(a.reshape(C, K, P, E).transpose(0, 2, 3, 1))  # [C,P,E,K]
        xrc = np.stack(parts, axis=3)                 # [C,P,E,2,K]
        maps.append(np.ascontiguousarray(xrc).reshape(C * P, 2 * EK))
    return maps


def _decode(outs):
    """Per-core [C*P, EK] u8 list -> [T, B, N] f32 spikes."""
    full = np.empty((T, B, N), np.float32)
    for i, o in enumerate(outs):
        a = o.reshape(C, P, E, K).transpose(0, 3, 1, 2)  # [C,K,P,E]
        a = a.reshape(T, BN)
        full[:, i * B_LOC:(i + 1) * B_LOC, :] = a.astype(np.float32).reshape(
            T, B_LOC, N
        )
    return full


LAST_RESULTS = None


def kernel(inp: np.ndarray, rec: np.ndarray, _trace=False, _tmpdir=None) -> np.ndarray:
    global LAST_RESULTS
    inp = np.asarray(inp, dtype=np.float32)
    rec = np.asarray(rec, dtype=np.float32)
    nc = _build_nc()
    xrmaps = _encode(inp, rec)
    in_maps = [{"xr": xrmaps[i]} for i in range(NCORES)]
    res = run_bass_kernel_spmd(nc, in_maps, list(range(NCORES)),
                               trace=_trace, tmpdir=_tmpdir)
    LAST_RESULTS = res
    return _decode([res.results[i]["s"] for i in range(NCORES)])
